# revision 10
# baseline (speedup 1.0000x reference)
"""Trainium2 Bass kernel for the DfOp deep-filtering module.

out[b, t, f<96]  = sum_{k=0..4} coefs[b, k, t, f] (*) spec[b, t-4+k, f]   (complex mult)
out[b, t, f>=96] = spec[b, t, f]                                          (passthrough)

Sharding: data-parallel over batch B=8 -> one batch element per NeuronCore.

The hi band (385 of 481 bins) is a pure passthrough, merged on the host
during gather; it never touches the device.  The device computes only the
96-bin lo band from HOST-PREPACKED planar fp16 (planes de-interleaved, im
coef plane pre-negated, causal halo prepacked per partition):

  spec  [2(piece), 2(plane), 128, 20*96]  piece 0 = window rows 0:20,
        piece 1 = rows 16:36 (4 rows duplicated so each 16-row product
        half-block reads exactly one piece -> whole-tile dependencies)
  coefs [5(tap), 2(half), 2(plane), 128, 16*96]   plane 1 = -c_im;
        the (re | -im) pair for one (tap, half) is contiguous -> ONE load
        and ONE completion semaphore feeds all 4 products of that tap
  consts [128, 256] = [I | -I] fp16 identity weights, single DMA
  out   [4096, 192] fp16   row t = [re(96) | im(96)], split on host

Schedule (from trace analysis):
  - exec = ~7us fixed preamble + ramp + dense DVE products + tail.
  - Ramp: first loads and tap-0 products split into 8-row chunks so the
    first tensor_tensor issues after ~600KB; sem waits on DVE are the
    hidden per-op cost, so loads are paired to minimize semaphores.
  - Mid: loads stream on the Sync ring in DVE consumption order; all
    products on DVE (Pool contends with DVE's SBUF ports; ACT cannot read
    two tensor streams).  PE accumulates via resident fp16 +/-identity
    into fp32 PSUM (512-col-max moving operand per matmul).
  - Tail: the last tap's products for half 1 run chunk 3 first, so chunk
    3's PSUM stop + drain + store overlap the chunk-2 products; the final
    chunk-2 drain is split ACT(re) || DVE(im).
"""

import sys

import numpy as np

try:
    import concourse.bacc  # noqa: F401  (resolves via the environment's path)
except ImportError:  # pragma: no cover - fallback for bare environments
    for _p in ("/opt/trn_rl_repo", "/root/.axon_site/_ro/trn_rl_repo"):
        if _p not in sys.path:
            sys.path.append(_p)

import concourse.bacc as bacc
import concourse.mybir as mybir
from concourse.tile import TileContext
from concourse.bass_utils import run_bass_kernel_spmd

B = 8          # batch / cores
T = 4096       # time steps
F = 481        # total freq bins
NF = 96        # deep-filtered freq bins
FS = 5         # frame size (causal taps)
HL = FS - 1    # halo slots (4)
P = 128        # partitions
TB = T // P    # timesteps per partition block   (32)
HALF = TB // 2                 # 16 rows per product half-block
SROWS = HALF + HL              # 20 rows per spec piece
WIN = TB + HL                  # 36 window rows (pieces at 0 and HALF)
CHUNK = 8                      # PSUM chunk rows
CW = CHUNK * NF                # 768 psum cols per re/im region
SC = SROWS * NF                # 1920 spec cols per plane
KC = HALF * NF                 # 1536 coef cols per plane

_nc_cache = None


def _body(nc, tc, spec_d, coef_d, const_d, out_d):
    f16 = mybir.dt.float16
    f32 = mybir.dt.float32

    outv = out_d.rearrange("(q i) u -> q i u", i=TB)            # [128, 32, 192]

    with (
        tc.tile_pool(name="const", bufs=1) as cpool,
        tc.tile_pool(name="spec", bufs=3) as spool,
        tc.tile_pool(name="coef", bufs=10) as kpool,
        tc.tile_pool(name="prod", bufs=26) as ppool,
        tc.tile_pool(name="out", bufs=4) as opool,
        tc.tile_pool(name="psum", bufs=2, space="PSUM") as pspool,
    ):
        # [I | -I] weights in one small DMA on the ACT ring so the Sync ring
        # starts on real data immediately
        cc = cpool.tile([P, 2 * P], f16)
        nc.scalar.dma_start(out=cc[:], in_=const_d)
        ident = cc[:, 0:P]
        identn = cc[:, P:2 * P]

        # piece-0 spec planes as separate tiles (ramp-chunked loads);
        # piece-1 spec planes paired in one tile / one load
        s0 = [spool.tile([P, SC], f16, tag="spec0", name=f"spec0{c}")
              for c in range(2)]
        s1 = spool.tile([P, 2 * SC], f16, tag="spec1", name="spec1")

        def spec_view(h, c):                                     # [P, 20, 96]
            ap = s0[c][:] if h == 0 else s1[:, c * SC:(c + 1) * SC]
            return ap.rearrange("p (i f) -> p i f", f=NF)

        # coef pair tiles: [P, re(1536) | -im(1536)] per (tap, half)
        ctiles = {}

        def make_coef(k, h):
            ct = kpool.tile([P, 2 * KC], f16, tag="coef", name=f"coef{k}{h}")
            ctiles[(k, h)] = ct
            return ct

        def load_coef(k, h, r0=0, r1=HALF):
            ct = ctiles[(k, h)]
            if r0 == 0 and r1 == HALF:
                dst = ct[:].rearrange("p (c f) -> p c f", c=2)
                src = coef_d[k, h].rearrange("c p f -> p c f")
            else:
                dst = ct[:].rearrange("p (c f) -> p c f", c=2)[:, :, r0 * NF:r1 * NF]
                src = coef_d[k, h][:, :, r0 * NF:r1 * NF].rearrange("c p f -> p c f")
            nc.sync.dma_start(out=dst, in_=src)

        def coef_view(k, h, c):                                  # [P, 16, 96]
            ap = ctiles[(k, h)][:, c * KC:(c + 1) * KC]
            return ap.rearrange("p (i f) -> p i f", f=NF)

        for k in range(FS):
            for h in range(2):
                make_coef(k, h)

        def load_spec0(c, r0, r1):
            nc.sync.dma_start(out=s0[c][:, r0 * NF:r1 * NF],
                              in_=spec_d[0, c][:, r0 * NF:r1 * NF])

        # loads (Sync ring, FIFO) in exact DVE consumption order
        load_spec0(0, 0, CHUNK)                                  # s_re 0:8
        load_coef(0, 0, 0, CHUNK)                                # c0 pair 0:8
        load_spec0(1, 0, CHUNK)                                  # s_im 0:8
        load_spec0(0, CHUNK, SROWS)                              # s_re 8:20
        load_spec0(1, CHUNK, SROWS)                              # s_im 8:20
        load_coef(0, 0, CHUNK, HALF)                             # c0 pair 8:16
        for k in range(1, FS):
            load_coef(k, 0)
        nc.sync.dma_start(out=s1[:].rearrange("p (c f) -> p c f", c=2),
                          in_=spec_d[1].rearrange("c p f -> p c f"))  # piece-1 pair
        for k in range(FS):
            load_coef(k, 1)

        # ---- products: all DVE ----
        # per (half h, tap k): rr = s_re*c_re, ir = s_im*c_re,
        #                      nr = s_re*(-c_im), ni = s_im*(-c_im)
        # re += rr + ni ; im += ir - nr  (via +/-I PSUM accumulation)
        prods = [[dict() for _ in range(FS)] for _ in range(2)]
        pv = lambda t: t[:].rearrange("p (i f) -> p i f", f=NF)

        def make_prods(h, k):
            prods[h][k] = {key: ppool.tile([P, KC], f16, tag="prod",
                                           name=f"prod{h}{k}{key}")
                           for key in ("rr", "ir", "nr", "ni")}

        def product(h, k, key, r0, r1):
            s_plane = 0 if key in ("rr", "nr") else 1
            c_plane = 0 if key in ("rr", "ir") else 1
            s = spec_view(h, s_plane)[:, k + r0:k + r1, :]
            c = coef_view(k, h, c_plane)[:, r0:r1, :]
            dst = pv(prods[h][k][key])[:, r0:r1, :]
            nc.vector.tensor_mul(out=dst, in0=s, in1=c)

        # ---- PE accumulation + drains ----
        pss = {}

        def make_psum(h):
            for ch in (2 * h, 2 * h + 1):
                pss[ch] = (
                    pspool.tile([P, CW], f32, tag="psre", name=f"psre{ch}"),
                    pspool.tile([P, CW], f32, tag="psim", name=f"psim{ch}"),
                )

        def mm(h, k, key, which, w, first, last, chunks=None):
            src = prods[h][k][key]
            for ch in chunks if chunks is not None else (2 * h, 2 * h + 1):
                off = (ch % 2) * CW
                ps = pss[ch][which]
                for a in range(0, CW, 512):
                    b = min(a + 512, CW)
                    nc.tensor.matmul(ps[:, a:b], w,
                                     src[:, off + a:off + b],
                                     start=first, stop=last)

        def mm_tap(h, k, first, last, chunks=None):
            mm(h, k, "rr", 0, ident, first, False, chunks)       # rr   -> re
            mm(h, k, "ir", 1, ident, first, False, chunks)       # ir   -> im
            mm(h, k, "ni", 0, ident, False, last, chunks)        # -ii  -> re
            mm(h, k, "nr", 1, identn, False, last, chunks)       # -ri  -> im

        def drain_store(ch, im_eng):
            ps_re, ps_im = pss[ch]
            ot = opool.tile([P, CHUNK * 2 * NF], f16, tag="out",
                            name=f"out{ch}")
            otv = ot[:].rearrange("p (i u) -> p i u", u=2 * NF)
            psv = lambda t: t[:].rearrange("p (i f) -> p i f", f=NF)
            nc.scalar.copy(out=otv[:, :, 0:NF], in_=psv(ps_re))
            if im_eng == "vector":
                nc.vector.tensor_copy(out=otv[:, :, NF:2 * NF],
                                      in_=psv(ps_im))
            else:
                nc.scalar.copy(out=otv[:, :, NF:2 * NF], in_=psv(ps_im))
            store_eng = nc.sync if ch >= 2 else nc.scalar
            store_eng.dma_start(
                out=outv[:, ch * CHUNK:(ch + 1) * CHUNK, :],
                in_=ot,
            )

        # ---- half 0: tap 0 in 8-row chunks (ramp), taps 1-4 full ----
        make_psum(0)
        make_prods(0, 0)
        for (r0, r1) in ((0, CHUNK), (CHUNK, HALF)):
            for key in ("rr", "ir", "nr", "ni"):
                product(0, 0, key, r0, r1)
        mm_tap(0, 0, True, False)
        for k in range(1, FS):
            make_prods(0, k)
            for key in ("rr", "ir", "nr", "ni"):
                product(0, k, key, 0, HALF)
            mm_tap(0, k, False, k == FS - 1)
        drain_store(0, "scalar")
        drain_store(1, "scalar")

        # ---- half 1: taps 0-3 full; tap 4 chunk 3 first, then chunk 2 ----
        make_psum(1)
        for k in range(FS - 1):
            make_prods(1, k)
            for key in ("rr", "ir", "nr", "ni"):
                product(1, k, key, 0, HALF)
            mm_tap(1, k, k == 0, False)
        k = FS - 1
        make_prods(1, k)
        for key in ("rr", "ir", "nr", "ni"):
            product(1, k, key, CHUNK, HALF)                      # chunk 3
        mm_tap(1, k, False, True, chunks=(3,))
        for key in ("rr", "ni", "ir", "nr"):
            product(1, k, key, 0, CHUNK)                         # chunk 2
        mm_tap(1, k, False, True, chunks=(2,))
        drain_store(3, "scalar")                                 # overlaps ch2 products
        drain_store(2, "vector")                                 # DVE idle after last TT


def _build_nc():
    nc = bacc.Bacc("TRN2", target_bir_lowering=False, debug=False, num_devices=B)
    f16 = mybir.dt.float16
    spec_d = nc.dram_tensor("spec", [2, 2, P, SC], f16,
                            kind="ExternalInput").ap()
    coef_d = nc.dram_tensor("coefs", [FS, 2, 2, P, KC], f16,
                            kind="ExternalInput").ap()
    const_d = nc.dram_tensor("consts", [P, 2 * P], f16,
                             kind="ExternalInput").ap()
    out_d = nc.dram_tensor("out", [T, 2 * NF], f16, kind="ExternalOutput").ap()
    with TileContext(nc) as tc:
        _body(nc, tc, spec_d, coef_d, const_d, out_d)
    nc.compile()
    return nc


def _in_maps(spec, coefs):
    spec = np.asarray(spec, dtype=np.float32)
    coefs = np.asarray(coefs, dtype=np.float32)
    consts = np.concatenate(
        [np.eye(P, dtype=np.float16), -np.eye(P, dtype=np.float16)], axis=1
    )
    maps = []
    for b in range(B):
        # spec window rows 32p-4..32p+32 per partition, then pieces
        # [0:20) and [16:36): [2(piece), 2(plane), 128, 20*96]
        s_lo = spec[b, 0, :, :NF, :].astype(np.float16)          # [4096, 96, 2]
        blk = s_lo.reshape(P, TB, NF, 2)
        win = np.zeros((P, WIN, NF, 2), dtype=np.float16)
        win[:, HL:] = blk
        win[1:, :HL] = blk[:-1, TB - HL:]
        wpl = win.transpose(3, 0, 1, 2)                          # [2,P,36,96]
        spec_pk = np.stack(
            [wpl[:, :, 0:SROWS], wpl[:, :, HALF:HALF + SROWS]], axis=0
        ).reshape(2, 2, P, SC)
        spec_pk = np.ascontiguousarray(spec_pk)

        # coefs: [5(tap), 2(half), 2(plane), 128, 16, 96]; plane 1 = -c_im
        c = coefs[b].reshape(FS, P, 2, HALF, NF, 2)
        cpk = c.transpose(0, 2, 5, 1, 3, 4).copy()  # [5,2(h),2(c),P,16,96]
        cpk[:, :, 1] *= -1.0
        coef_pk = cpk.astype(np.float16).reshape(FS, 2, 2, P, KC)

        maps.append({"spec": spec_pk, "coefs": coef_pk, "consts": consts})
    return maps


def kernel(spec, coefs):
    global _nc_cache
    if _nc_cache is None:
        _nc_cache = _build_nc()
    res = run_bass_kernel_spmd(_nc_cache, _in_maps(spec, coefs),
                               core_ids=list(range(B)))
    out = np.asarray(spec, dtype=np.float32).copy()              # hi band
    for b in range(B):
        lo = res.results[b]["out"].astype(np.float32)            # [4096, 192]
        out[b, 0, :, :NF, 0] = lo[:, :NF]
        out[b, 0, :, :NF, 1] = lo[:, NF:]
    return out
